# revision 30
# baseline (speedup 1.0000x reference)
"""MatchingNet head (cosine-sim kNN aggregation) on 8 trn2 NeuronCores.

Reference computation:
    sim[m, n] = <fX[m], gS[n]> / max(||fX[m]|| * ||gS[n]||, 1e-8)
    out[m, c] = sum_n sim[m, n] * onehot(trainTarget)[n, c]

Exact algebraic reassociation (the eps guard never binds for D=1024 randn
rows, whose norms concentrate around 32):
    A = gS.T @ (onehot / ||gS||)          # [D, C]
    out = diag(1/||fX||) @ (fX @ A)        # [M, C]

Two SPMD launches (on-device collectives work functionally here but the
~32us core-dispatch skew + ~34us first-collective rendezvous make any
cross-core sync slower than a second launch):
  Phase 1: gS sharded row-wise; core i computes the partial
           A_i.T = (onehot_i / ||gS_i||).T @ gS_i  over its 512 supports.
           The host sums the eight [64, 1024] fp32 partials and retiles
           the bf16 A for phase 2.
  Phase 2: fX sharded row-wise; each core streams fX.T by 128-dim chunks
           through the PE (A chunk stationary), accumulates OT[c, m] in
           PSUM, reduces per-query sum-of-squares with all-ones [128, 64]
           stationary matmuls (which broadcast to the 64 output
           partitions), applies 1/sqrt via Abs_reciprocal_sqrt, and
           scales OT during the PSUM->SBUF drain.

Schedule notes (from perfetto traces of the previous build):
  - PE clock ramps 1.2->2.4 GHz only after ~5.5us of sustained work and
    decays in gaps; phase 2 front-pads junk matmuls sized to end at
    chunk 0's arrival so the real stream runs mostly at the fast rate.
  - All DMAs use flat 2D access patterns (the 3D rearrange APs cost
    ~1.5us of DIRECT2D descriptor generation before any byte moves).
  - Phase 2 DMAs fxt so it is fully resident early; the norm reduction
    (squares, adds, PSUM folds, rsqrt) completes mid-stream and the only
    work after the last sim matmul is the final scale + output DMA.
    (The old build DMA'd chunk 7 last and paid a ~5us norm tail.)
  - 1/sqrt(h0+h1) is fused into one ACT op via the bias operand.

All matmul operands are bf16 (PSUM accumulation stays fp32). Norm
squares are fp16 (bf16's 8-bit mantissa loses ~1% on the chunk-sum
accumulation; fp16's 11 bits keep it ~1e-4 and the sums stay < 2048).
"""

import numpy as np
from contextlib import ExitStack

import ml_dtypes

import concourse.bass as bass  # noqa: F401
import concourse.tile as tile
import concourse.mybir as mybir
from concourse import bacc, bass2jax
from concourse.bass_utils import run_bass_kernel_spmd

N, D, C, M = 4096, 1024, 64, 8192
NCORES = 8
NS = N // NCORES   # 512 supports per core (phase 1)
MS = M // NCORES   # 1024 queries per core (phase 2)
P = 128
NT = NS // P       # 4 n-tiles per core
DC = D // P        # 8 d-chunks (128 each)
HB = 512           # half width (one PSUM bank of fp32)
F32 = mybir.dt.float32
BF16 = mybir.dt.bfloat16
F16 = mybir.dt.float16
AF = mybir.ActivationFunctionType
MUL = mybir.AluOpType.mult
BF16NP = ml_dtypes.bfloat16

# phase-2 PE warm-up matmuls (each ~0.6us cold); sized to end ~ when
# chunk 0 lands so they never delay a real matmul on the in-order PE.
P2_JUNK = 14

_CACHE = {}


P1_JUNK = 18


def _build_phase1():
    nc = bacc.Bacc(
        "TRN2", target_bir_lowering=False, debug=False, num_devices=NCORES
    )
    gs = nc.dram_tensor("gs", [P, NT * D], BF16, kind="ExternalInput").ap()
    oh = nc.dram_tensor("oh", [P, NT * C], BF16, kind="ExternalInput").ap()
    atp = nc.dram_tensor("atp", [C, D], BF16, kind="ExternalOutput").ap()
    wrm = nc.dram_tensor("wrm", [16, D], BF16, kind="Internal").ap()

    with tile.TileContext(nc) as tc, ExitStack() as ctx:
        const_pool = ctx.enter_context(tc.tile_pool(name="const", bufs=1))
        work_pool = ctx.enter_context(tc.tile_pool(name="wrk", bufs=2))
        psA = ctx.enter_context(tc.tile_pool(name="psA", bufs=1, space="PSUM"))

        gs_sb = const_pool.tile([P, NT * D], BF16, tag="gs")
        oh_sb = const_pool.tile([P, NT * C], BF16, tag="oh")
        # Baseline-measured DMA plan: per-tile singles (2KB lines) paced
        # ~0.8us apart; oh leads on the scalar queue.
        # 32KB DRAM->DRAM ring warmer (16 descs, one per ring): the first
        # ~256KB of a cold stream crawls at ~80GB/s; this probes whether
        # early ring activity shortens that ramp without SBUF traffic.
        nc.sync.dma_start(wrm[:, :], gs[0:16, 0:D])
        nc.scalar.dma_start(oh_sb[:], oh[:, :])
        nc.sync.dma_start(gs_sb[:, 0:D], gs[:, 0:D])
        nc.scalar.dma_start(gs_sb[:, D:2 * D], gs[:, D:2 * D])
        nc.sync.dma_start(gs_sb[:, 2 * D:3 * D], gs[:, 2 * D:3 * D])
        nc.scalar.dma_start(gs_sb[:, 3 * D:4 * D], gs[:, 3 * D:4 * D])
        # Preload BOTH activation tables (Square + Abs_rsqrt) before the
        # first data-dependent ACT op; a mid-stream table swap costs
        # ~1.3us on the ACT critical path. No PE warm-up junk: its
        # moving-operand SBUF reads tax the DMA stream ~0.45us per
        # matmul (measured), a net loss in this DMA-bound phase.
        with tc.high_priority():
            dumm = const_pool.tile([1, 1], F32, tag="dumm")
            nc.gpsimd.memset(dumm[:], 1.0)
            dumm2 = const_pool.tile([1, 1], F32, tag="dumm2")
            nc.scalar.activation(dumm2[:], dumm[:], AF.Square)
            nc.scalar.activation(dumm2[:], dumm[:], AF.Abs_reciprocal_sqrt)
            # Narrow PE keep-alive matmuls: the chip clock drops ~1.2x on
            # all engines when the PE idles, but wide junk's moving reads
            # starve the DMA stream's SBUF writes. [128,1]x[128,64] keeps
            # the PE busy at ~8KB SBUF read per matmul (53ns each).
            jmov = const_pool.tile([P, C], BF16, tag="jmov")
            nc.gpsimd.memset(jmov[:], 1.0)
            pj = psA.tile([1, C], F32, tag="junkp", name="pj")
            for i in range(P1_JUNK):
                nc.tensor.matmul(pj[:], jmov[:, 0:1], jmov[:], start=True,
                                 stop=True)

        pa = [psA.tile([C, HB], F32, tag=f"at{h}", name=f"pa{h}")
              for h in range(2)]
        gsq = [
            [const_pool.tile([P, 1], F32, tag=f"gsq{t}h{h}",
                             name=f"gsq{t}h{h}") for h in range(2)]
            for t in range(NT)
        ]
        grinv = [
            const_pool.tile([P, 1], F32, tag=f"gr{t}", name=f"gr{t}")
            for t in range(NT)
        ]
        wt = [
            const_pool.tile([P, C], BF16, tag=f"w{t}", name=f"w{t}")
            for t in range(NT)
        ]

        def seg(t, h=None):
            if h is None:
                return gs_sb[:, t * D:(t + 1) * D]
            return gs_sb[:, t * D + h * HB:t * D + (h + 1) * HB]

        def stt_sq(t, h, gout):
            sqt = work_pool.tile([P, D if h is None else HB], BF16,
                                 tag="sq", name=f"sq{t}{h}")
            nc.vector.scalar_tensor_tensor(
                out=sqt[:], in0=seg(t, h), scalar=1.0, in1=seg(t, h),
                op0=MUL, op1=MUL, accum_out=gout,
            )

        def act_sq(t, h, gout):
            sqt = work_pool.tile([P, D if h is None else HB], BF16,
                                 tag="sq", name=f"sqa{t}{h}")
            nc.scalar.activation(sqt[:], seg(t, h), AF.Square,
                                 accum_out=gout)

        # Every tile's square runs split h0->DVE / h1->ACT so the two
        # halves proceed in parallel (~0.6us vs ~1.2us full-width) and
        # 1/sqrt(h0+h1) fuses the combine via the ACT bias operand.
        # wt scaling on DVE except the last tile (keeps its whole tail
        # on ACT right after ars3).
        with tc.high_priority(offset=1):
            stt_sq(0, None, gsq[0][0][:])          # DVE
            nc.scalar.activation(grinv[0][:], gsq[0][0][:],
                                 AF.Abs_reciprocal_sqrt)
            nc.vector.tensor_scalar_mul(wt[0][:], oh_sb[:, 0:C],
                                        grinv[0][:])
        act_sq(1, None, gsq[1][0][:])              # ACT
        stt_sq(2, None, gsq[2][0][:])              # DVE
        nc.scalar.activation(grinv[1][:], gsq[1][0][:],
                             AF.Abs_reciprocal_sqrt)
        nc.scalar.activation(wt[1][:], oh_sb[:, C:2 * C], AF.Copy,
                             scale=grinv[1][:])
        stt_sq(3, None, gsq[3][0][:])              # DVE
        nc.scalar.activation(grinv[2][:], gsq[2][0][:],
                             AF.Abs_reciprocal_sqrt)
        nc.vector.tensor_scalar_mul(wt[2][:], oh_sb[:, 2 * C:3 * C],
                                    grinv[2][:])
        nc.scalar.activation(grinv[3][:], gsq[3][0][:],
                             AF.Abs_reciprocal_sqrt)
        nc.scalar.activation(wt[3][:], oh_sb[:, 3 * C:4 * C], AF.Copy,
                             scale=grinv[3][:])

        for t in range(NT):
            for h in range(2):
                nc.tensor.matmul(
                    pa[h][:],
                    wt[t][:],
                    seg(t, h),
                    start=(t == 0),
                    stop=(t == NT - 1),
                )
        # Drain in halves; pa is split per-half so the h0 copy does not
        # falsely wait on the h1 matmul (whole-tile dep tracking).
        o = const_pool.tile([C, D], BF16, tag="o")
        QB1 = HB // 2
        for h in range(2):
            for q in range(2):
                qs = slice(h * HB + q * QB1, h * HB + (q + 1) * QB1)
                qp = slice(q * QB1, (q + 1) * QB1)
                nc.vector.tensor_copy(o[:, qs], pa[h][:, qp])
                queue = nc.sync if h == 0 else nc.scalar
                queue.dma_start(atp[:, qs], o[:, qs])

    nc.compile()
    return nc


def _build_phase2():
    nc = bacc.Bacc(
        "TRN2", target_bir_lowering=False, debug=False, num_devices=NCORES
    )
    a = nc.dram_tensor("a", [P, DC * C], BF16, kind="ExternalInput").ap()
    fxt = nc.dram_tensor("fxt", [P, DC * MS], BF16, kind="ExternalInput").ap()
    out = nc.dram_tensor("out", [C, MS], BF16, kind="ExternalOutput").ap()
    wrm = nc.dram_tensor("wrm", [16, MS], BF16, kind="Internal").ap()

    with tile.TileContext(nc) as tc, ExitStack() as ctx:
        const_pool = ctx.enter_context(tc.tile_pool(name="const", bufs=1))
        sq_pool = ctx.enter_context(tc.tile_pool(name="sqp", bufs=4))
        st_pool = const_pool
        os_pool = const_pool
        psO = ctx.enter_context(tc.tile_pool(name="psO", bufs=1, space="PSUM"))
        psF = psO
        psJ = psO

        a_sb = const_pool.tile([P, DC * C], BF16, tag="a")
        fxt_sb = const_pool.tile([P, DC * MS], BF16, tag="fxt")

        def chunk_ap(k):
            return fxt_sb[:, k * MS:(k + 1) * MS]

        def pair_dma(q, k0):
            q.dma_start(
                fxt_sb[:, k0 * MS:(k0 + 2) * MS].rearrange(
                    "p (t m) -> p t m", t=2),
                fxt[:, k0 * MS:(k0 + 2) * MS].rearrange(
                    "p (t m) -> p t m", t=2),
            )

        # Baseline-measured DMA byte schedule (singles + 3D pairs +
        # trailing halves), with the chunk labels arranged so the LAST
        # arrival is c6 (direct-folded, split-squared) instead of c7:
        #   sync:   c0, [c1 c2], [c3 c4], c6h0, c6h1
        #   scalar: a, c5, c7
        # 32KB DRAM->DRAM ring warmer (see phase 1).
        nc.sync.dma_start(wrm[:, :], fxt[0:16, 0:MS])
        nc.sync.dma_start(chunk_ap(0), fxt[:, 0:MS])
        pair_dma(nc.sync, 1)          # c1, c2
        nc.scalar.dma_start(chunk_ap(7), fxt[:, 7 * MS:8 * MS])
        pair_dma(nc.sync, 3)          # c3, c4
        nc.scalar.dma_start(a_sb[:], a[:, :])
        nc.scalar.dma_start(chunk_ap(5), fxt[:, 5 * MS:6 * MS])
        nc.sync.dma_start(fxt_sb[:, 6 * MS:6 * MS + HB],
                          fxt[:, 6 * MS:6 * MS + HB])
        nc.sync.dma_start(fxt_sb[:, 6 * MS + HB:7 * MS],
                          fxt[:, 6 * MS + HB:7 * MS])

        ones_sb = const_pool.tile([P, C], F16, tag="ones")
        nc.gpsimd.memset(ones_sb[:], 1.0)
        with tc.high_priority():
            dumm = st_pool.tile([1, 1], F32, tag="dumm")
            nc.gpsimd.memset(dumm[:], 1.0)
            dumm2 = st_pool.tile([1, 1], F32, tag="dumm2")
            nc.scalar.activation(dumm2[:], dumm[:], AF.Square)
            nc.scalar.activation(dumm2[:], dumm[:], AF.Abs_reciprocal_sqrt)
            # Narrow PE keep-alive matmuls (see phase 1).
            jmov = const_pool.tile([P, C], BF16, tag="jmov")
            nc.gpsimd.memset(jmov[:], 1.0)
            pj = psJ.tile([1, C], F32, tag="junkp", name="pj")
            for i in range(P2_JUNK):
                nc.tensor.matmul(pj[:], jmov[:, 0:1], jmov[:], start=True,
                                 stop=True)

        # po/pf split per 512-wide half (separate PSUM banks) so reads
        # of one half never falsely serialize against writes of the
        # other (whole-tile dep tracking).
        po = [psO.tile([C, HB], F32, tag=f"ot{h}", name=f"po{h}")
              for h in range(2)]
        pf = [psF.tile([C, HB], F32, tag=f"fs{h}", name=f"pf{h}")
              for h in range(2)]
        # Square accumulators grouped by arrival: sacc0={0,1,2} and
        # sacc1={3,4} (adds on DVE), sacc2={5,7} (add on DVE). The
        # last-arriving chunk c6 skips accumulation: its square runs as
        # h0->DVE / h1->ACT in parallel and folds straight into pf.
        sacc = [
            const_pool.tile([P, MS], F16, tag=f"sacc{j}", name=f"sacc{j}")
            for j in range(2)
        ]
        GROUP = {0: 0, 1: 0, 2: 0, 3: 0, 4: 1, 5: 1}
        INIT = {0: True, 4: True}
        ACT_SQ = (1, 3, 5)
        SQ_ORDER = (0, 1, 2, 3, 4, 5)
        for k in SQ_ORDER:
            acc = sacc[GROUP[k]]
            dst = acc if INIT.get(k) else sq_pool.tile(
                [P, MS], F16, tag="sq", name=f"sq{k}")
            if k in ACT_SQ:
                nc.scalar.activation(dst[:], chunk_ap(k), AF.Square)
            else:
                nc.vector.tensor_mul(dst[:], chunk_ap(k), chunk_ap(k))
            if not INIT.get(k):
                nc.vector.tensor_add(acc[:], acc[:], dst[:])
        # c7 and c6 skip the accumulator: their squares fold straight
        # into pf, so no serial DVE add sits between their arrival and
        # the fold. c7 squares on ACT (it arrives early, scalar-first);
        # c6 (last arrival) splits h0->DVE / h1->ACT.
        sq7 = sq_pool.tile([P, MS], F16, tag="sq7", name="sq7")
        nc.scalar.activation(sq7[:], chunk_ap(7), AF.Square)
        sq6 = sq_pool.tile([P, MS], F16, tag="sq6", name="sq6")
        nc.vector.tensor_mul(sq6[:, 0:HB], chunk_ap(6)[:, 0:HB],
                             chunk_ap(6)[:, 0:HB])
        nc.scalar.activation(sq6[:, HB:MS], chunk_ap(6)[:, HB:MS],
                             AF.Square)

        # Sim stream in order 0..5,7,6 (PSUM accumulation commutes); the
        # sacc folds interleave at the k==5/k==7 slots and c6's direct
        # fold is the only norm work after the last sim matmul.
        frinv = const_pool.tile([C, MS], F32, tag="frinv")
        ot_sb = os_pool.tile([C, MS], BF16, tag="otsb")
        MM_ORDER = (0, 1, 2, 3, 4, 5, 7, 6)
        for i, k in enumerate(MM_ORDER):
            chunk = chunk_ap(k)
            for h in range(2):
                nc.tensor.matmul(
                    po[h][:],
                    a_sb[:, k * C:(k + 1) * C],
                    chunk[:, h * HB:(h + 1) * HB],
                    start=(i == 0),
                    stop=(i == DC - 1),
                )
            if k == 5:
                for h in range(2):
                    hs = slice(h * HB, (h + 1) * HB)
                    nc.tensor.matmul(pf[h][:], ones_sb[:], sacc[0][:, hs],
                                     start=True, stop=False)
            if k == 7:
                for h in range(2):
                    hs = slice(h * HB, (h + 1) * HB)
                    nc.tensor.matmul(pf[h][:], ones_sb[:], sq7[:, hs],
                                     start=False, stop=False)
        for h in range(2):
            hs = slice(h * HB, (h + 1) * HB)
            nc.tensor.matmul(pf[h][:], ones_sb[:], sacc[1][:, hs],
                             start=False, stop=False)
        QB = HB // 2
        for h in range(2):
            hs = slice(h * HB, (h + 1) * HB)
            nc.tensor.matmul(pf[h][:], ones_sb[:], sq6[:, hs],
                             start=False, stop=True)
            for q in range(2):
                qs = slice(h * HB + q * QB, h * HB + (q + 1) * QB)
                qp = slice(q * QB, (q + 1) * QB)
                nc.scalar.activation(frinv[:, qs], pf[h][:, qp],
                                     AF.Abs_reciprocal_sqrt)
                nc.vector.tensor_mul(ot_sb[:, qs], po[h][:, qp],
                                     frinv[:, qs])
                queue = nc.sync if h == 0 else nc.scalar
                queue.dma_start(out[:, qs], ot_sb[:, qs])

    nc.compile()
    return nc


def _get_ncs():
    if "nc1" not in _CACHE:
        _CACHE["nc1"] = _build_phase1()
        _CACHE["nc2"] = _build_phase2()
    return _CACHE["nc1"], _CACHE["nc2"]


class _FakeResult:
    def __init__(self, results):
        self.results = results
        self.exec_time_ns = None
        self.instructions_and_trace = None


def _make_runner(nc):
    """One persistently-jitted shard_map executable for this Bass module.

    run_bass_via_pjrt rebuilds its jit closure per call, which retraces and
    re-lowers the HLO every invocation (~3 s/launch of host time). Caching
    the jitted callable keeps warmed kernel() calls fast; the device-side
    NEFF and its execution are identical.
    """
    import jax
    import numpy as _np

    bass2jax.install_neuronx_cc_hook()
    Mesh = bass2jax.Mesh
    PartitionSpec = bass2jax.PartitionSpec
    shard_map = bass2jax.shard_map

    partition_name = (
        nc.partition_id_tensor.name if nc.partition_id_tensor else None
    )
    in_names, out_names, out_avals, zero_shapes = [], [], [], []
    for alloc in nc.m.functions[0].allocations:
        if not isinstance(alloc, mybir.MemoryLocationSet):
            continue
        name = alloc.memorylocations[0].name
        if alloc.kind == "ExternalInput":
            if name != partition_name:
                in_names.append(name)
        elif alloc.kind == "ExternalOutput":
            shape = tuple(alloc.tensor_shape)
            dtype = mybir.dt.np(alloc.dtype)
            out_avals.append(jax.core.ShapedArray(shape, dtype))
            out_names.append(name)
            zero_shapes.append((shape, dtype))
    n_params = len(in_names)
    all_in = list(in_names) + list(out_names)
    if partition_name is not None:
        all_in.append(partition_name)
    donate = tuple(range(n_params, n_params + len(out_names)))

    def _body(*args):
        operands = list(args)
        if partition_name is not None:
            operands.append(bass2jax.partition_id_tensor())
        outs = bass2jax._bass_exec_p.bind(
            *operands,
            out_avals=tuple(out_avals),
            in_names=tuple(all_in),
            out_names=tuple(out_names),
            lowering_input_output_aliases=(),
            sim_require_finite=True,
            sim_require_nnan=True,
            nc=nc,
        )
        return tuple(outs)

    devices = jax.devices()[:NCORES]
    mesh = Mesh(_np.asarray(devices), ("core",))
    nspec = n_params + len(out_names)
    sharded = jax.jit(
        shard_map(
            _body, mesh=mesh,
            in_specs=(PartitionSpec("core"),) * nspec,
            out_specs=(PartitionSpec("core"),) * len(out_names),
            check_rep=False,
        ),
        donate_argnums=donate,
        keep_unused=True,
    )

    def runner(in_maps):
        concat_in = [
            _np.concatenate([_np.asarray(m[name]) for m in in_maps], axis=0)
            for name in in_names
        ]
        concat_zeros = [
            _np.zeros((NCORES * s[0], *s[1:]), dt) for s, dt in zero_shapes
        ]
        out_arrs = sharded(*concat_in, *concat_zeros)
        return _FakeResult([
            {
                name: _np.asarray(out_arrs[i]).reshape(
                    NCORES, *out_avals[i].shape
                )[c]
                for i, name in enumerate(out_names)
            }
            for c in range(NCORES)
        ])

    return runner


def _get_runners():
    if "run1" not in _CACHE:
        nc1, nc2 = _get_ncs()
        _CACHE["run1"] = _make_runner(nc1)
        _CACHE["run2"] = _make_runner(nc2)
    return _CACHE["run1"], _CACHE["run2"]


def _tile_rows_flat(arr, ntiles):
    """[ntiles*128, F] -> [128, ntiles*F] with [p, t*F+f] = arr[t*128+p, f]."""
    f = arr.shape[1]
    return np.ascontiguousarray(
        arr.reshape(ntiles, P, f).transpose(1, 0, 2).reshape(P, ntiles * f)
    )


def run(gS, fX, trainTarget, nClasses, trace=False, **spmd_kwargs):
    nc1, nc2 = _get_ncs()
    gS = np.asarray(gS, dtype=np.float32).astype(BF16NP)
    fX = np.asarray(fX, dtype=np.float32).astype(BF16NP)
    tt = np.asarray(trainTarget).astype(np.int64).ravel()
    nc_classes = int(np.asarray(nClasses))
    assert nc_classes == C and gS.shape == (N, D) and fX.shape == (M, D)

    oh = np.zeros((N, C), dtype=BF16NP)
    oh[np.arange(N), tt] = 1.0

    in_maps1 = []
    for i in range(NCORES):
        gsl = gS[i * NS:(i + 1) * NS]
        osl = oh[i * NS:(i + 1) * NS]
        in_maps1.append(
            {"gs": _tile_rows_flat(gsl, NT), "oh": _tile_rows_flat(osl, NT)}
        )
    if trace or spmd_kwargs:
        res1 = run_bass_kernel_spmd(
            nc1, in_maps1, core_ids=list(range(NCORES)), trace=trace,
            **spmd_kwargs
        )
    else:
        res1 = _get_runners()[0](in_maps1)
    # gather-reduce the partial A.T's, retile to [128, 8*64] bf16
    at = np.zeros((C, D), dtype=np.float32)
    for i in range(NCORES):
        at += res1.results[i]["atp"].astype(np.float32)
    a_tiled = _tile_rows_flat(np.ascontiguousarray(at.T.astype(BF16NP)), DC)

    in_maps2 = []
    for i in range(NCORES):
        sl = fX[i * MS:(i + 1) * MS]                       # [MS, D] bf16
        fxt_tiled = np.ascontiguousarray(
            sl.T.reshape(DC, P, MS).transpose(1, 0, 2).reshape(P, DC * MS)
        )
        in_maps2.append({"a": a_tiled, "fxt": fxt_tiled})
    if trace or spmd_kwargs:
        res2 = run_bass_kernel_spmd(
            nc2, in_maps2, core_ids=list(range(NCORES)), trace=trace,
            **spmd_kwargs
        )
    else:
        res2 = _get_runners()[1](in_maps2)
    outs = [
        np.ascontiguousarray(res2.results[i]["out"].T).astype(np.float32)
        for i in range(NCORES)
    ]
    full = np.concatenate(outs, axis=0)
    return full, (res1, res2)


def _axon_reset():
    """Recover a wedged exec unit (NRT_EXEC_UNIT_UNRECOVERABLE) left over
    from an earlier crashed run; no-op on a healthy device."""
    try:
        import ctypes
        import jax

        lib = ctypes.CDLL("/opt/axon/libaxon_pjrt.so")
        lib.axon_reset.restype = ctypes.c_int64
        jax.devices()
        lib.axon_reset(ctypes.c_int32(0))
        jax.clear_caches()
        _CACHE.pop("run1", None)
        _CACHE.pop("run2", None)
    except Exception:
        pass


def kernel(gS, fX, trainTarget, nClasses):
    try:
        full, _ = run(gS, fX, trainTarget, nClasses)
    except Exception:
        _axon_reset()
        full, _ = run(gS, fX, trainTarget, nClasses)
    return full


# revision 31
# speedup vs baseline: 1.0144x; 1.0144x over previous
"""MatchingNet head (cosine-sim kNN aggregation) on 8 trn2 NeuronCores.

Reference computation:
    sim[m, n] = <fX[m], gS[n]> / max(||fX[m]|| * ||gS[n]||, 1e-8)
    out[m, c] = sum_n sim[m, n] * onehot(trainTarget)[n, c]

Exact algebraic reassociation (the eps guard never binds for D=1024 randn
rows, whose norms concentrate around 32):
    A = gS.T @ (onehot / ||gS||)          # [D, C]
    out = diag(1/||fX||) @ (fX @ A)        # [M, C]

Two SPMD launches (on-device collectives work functionally here but the
~32us core-dispatch skew + ~34us first-collective rendezvous make any
cross-core sync slower than a second launch):
  Phase 1: gS sharded row-wise; core i computes the partial
           A_i.T = (onehot_i / ||gS_i||).T @ gS_i  over its 512 supports.
           The host sums the eight [64, 1024] fp32 partials and retiles
           the bf16 A for phase 2.
  Phase 2: fX sharded row-wise; each core streams fX.T by 128-dim chunks
           through the PE (A chunk stationary), accumulates OT[c, m] in
           PSUM, reduces per-query sum-of-squares with all-ones [128, 64]
           stationary matmuls (which broadcast to the 64 output
           partitions), applies 1/sqrt via Abs_reciprocal_sqrt, and
           scales OT during the PSUM->SBUF drain.

Schedule notes (from perfetto traces of the previous build):
  - PE clock ramps 1.2->2.4 GHz only after ~5.5us of sustained work and
    decays in gaps; phase 2 front-pads junk matmuls sized to end at
    chunk 0's arrival so the real stream runs mostly at the fast rate.
  - All DMAs use flat 2D access patterns (the 3D rearrange APs cost
    ~1.5us of DIRECT2D descriptor generation before any byte moves).
  - Phase 2 DMAs fxt so it is fully resident early; the norm reduction
    (squares, adds, PSUM folds, rsqrt) completes mid-stream and the only
    work after the last sim matmul is the final scale + output DMA.
    (The old build DMA'd chunk 7 last and paid a ~5us norm tail.)
  - 1/sqrt(h0+h1) is fused into one ACT op via the bias operand.

All matmul operands are bf16 (PSUM accumulation stays fp32). Norm
squares are fp16 (bf16's 8-bit mantissa loses ~1% on the chunk-sum
accumulation; fp16's 11 bits keep it ~1e-4 and the sums stay < 2048).
"""

import numpy as np
from contextlib import ExitStack

import ml_dtypes

import concourse.bass as bass  # noqa: F401
import concourse.tile as tile
import concourse.mybir as mybir
from concourse import bacc, bass2jax
from concourse.bass_utils import run_bass_kernel_spmd

N, D, C, M = 4096, 1024, 64, 8192
NCORES = 8
NS = N // NCORES   # 512 supports per core (phase 1)
MS = M // NCORES   # 1024 queries per core (phase 2)
P = 128
NT = NS // P       # 4 n-tiles per core
DC = D // P        # 8 d-chunks (128 each)
HB = 512           # half width (one PSUM bank of fp32)
F32 = mybir.dt.float32
BF16 = mybir.dt.bfloat16
F16 = mybir.dt.float16
AF = mybir.ActivationFunctionType
MUL = mybir.AluOpType.mult
BF16NP = ml_dtypes.bfloat16

# phase-2 PE warm-up matmuls (each ~0.6us cold); sized to end ~ when
# chunk 0 lands so they never delay a real matmul on the in-order PE.
P2_JUNK = 14

_CACHE = {}


P1_JUNK = 18


def _build_phase1():
    nc = bacc.Bacc(
        "TRN2", target_bir_lowering=False, debug=False, num_devices=NCORES
    )
    gs = nc.dram_tensor("gs", [P, NT * D], BF16, kind="ExternalInput").ap()
    oh = nc.dram_tensor("oh", [P, NT * C], BF16, kind="ExternalInput").ap()
    atp = nc.dram_tensor("atp", [C, D], BF16, kind="ExternalOutput").ap()

    with tile.TileContext(nc) as tc, ExitStack() as ctx:
        const_pool = ctx.enter_context(tc.tile_pool(name="const", bufs=1))
        work_pool = ctx.enter_context(tc.tile_pool(name="wrk", bufs=2))
        psA = ctx.enter_context(tc.tile_pool(name="psA", bufs=1, space="PSUM"))

        gs_sb = const_pool.tile([P, NT * D], BF16, tag="gs")
        oh_sb = const_pool.tile([P, NT * C], BF16, tag="oh")
        # Baseline-measured DMA plan: per-tile singles (2KB lines) paced
        # ~0.8us apart; oh leads on the scalar queue.
        nc.scalar.dma_start(oh_sb[:], oh[:, :])
        nc.sync.dma_start(gs_sb[:, 0:D], gs[:, 0:D])
        nc.scalar.dma_start(gs_sb[:, D:2 * D], gs[:, D:2 * D])
        nc.sync.dma_start(gs_sb[:, 2 * D:3 * D], gs[:, 2 * D:3 * D])
        nc.scalar.dma_start(gs_sb[:, 3 * D:4 * D], gs[:, 3 * D:4 * D])
        # Preload BOTH activation tables (Square + Abs_rsqrt) before the
        # first data-dependent ACT op; a mid-stream table swap costs
        # ~1.3us on the ACT critical path. No PE warm-up junk: its
        # moving-operand SBUF reads tax the DMA stream ~0.45us per
        # matmul (measured), a net loss in this DMA-bound phase.
        with tc.high_priority():
            dumm = const_pool.tile([1, 1], F32, tag="dumm")
            nc.gpsimd.memset(dumm[:], 1.0)
            dumm2 = const_pool.tile([1, 1], F32, tag="dumm2")
            nc.scalar.activation(dumm2[:], dumm[:], AF.Square)
            nc.scalar.activation(dumm2[:], dumm[:], AF.Abs_reciprocal_sqrt)
            # Narrow PE keep-alive matmuls: the chip clock drops ~1.2x on
            # all engines when the PE idles, but wide junk's moving reads
            # starve the DMA stream's SBUF writes. [128,1]x[128,64] keeps
            # the PE busy at ~8KB SBUF read per matmul (53ns each).
            jmov = const_pool.tile([P, C], BF16, tag="jmov")
            nc.gpsimd.memset(jmov[:], 1.0)
            pj = psA.tile([1, C], F32, tag="junkp", name="pj")
            for i in range(P1_JUNK):
                nc.tensor.matmul(pj[:], jmov[:, 0:1], jmov[:], start=True,
                                 stop=True)

        pa = [psA.tile([C, HB], F32, tag=f"at{h}", name=f"pa{h}")
              for h in range(2)]
        gsq = [
            [const_pool.tile([P, 1], F32, tag=f"gsq{t}h{h}",
                             name=f"gsq{t}h{h}") for h in range(2)]
            for t in range(NT)
        ]
        grinv = [
            const_pool.tile([P, 1], F32, tag=f"gr{t}", name=f"gr{t}")
            for t in range(NT)
        ]
        wt = [
            const_pool.tile([P, C], BF16, tag=f"w{t}", name=f"w{t}")
            for t in range(NT)
        ]

        def seg(t, h=None):
            if h is None:
                return gs_sb[:, t * D:(t + 1) * D]
            return gs_sb[:, t * D + h * HB:t * D + (h + 1) * HB]

        def stt_sq(t, h, gout):
            sqt = work_pool.tile([P, D if h is None else HB], BF16,
                                 tag="sq", name=f"sq{t}{h}")
            nc.vector.scalar_tensor_tensor(
                out=sqt[:], in0=seg(t, h), scalar=1.0, in1=seg(t, h),
                op0=MUL, op1=MUL, accum_out=gout,
            )

        def act_sq(t, h, gout):
            sqt = work_pool.tile([P, D if h is None else HB], BF16,
                                 tag="sq", name=f"sqa{t}{h}")
            nc.scalar.activation(sqt[:], seg(t, h), AF.Square,
                                 accum_out=gout)

        # Every tile's square runs split h0->DVE / h1->ACT so the two
        # halves proceed in parallel (~0.6us vs ~1.2us full-width) and
        # 1/sqrt(h0+h1) fuses the combine via the ACT bias operand.
        # wt scaling on DVE except the last tile (keeps its whole tail
        # on ACT right after ars3).
        with tc.high_priority(offset=1):
            stt_sq(0, None, gsq[0][0][:])          # DVE
            nc.scalar.activation(grinv[0][:], gsq[0][0][:],
                                 AF.Abs_reciprocal_sqrt)
            nc.vector.tensor_scalar_mul(wt[0][:], oh_sb[:, 0:C],
                                        grinv[0][:])
        act_sq(1, None, gsq[1][0][:])              # ACT
        stt_sq(2, None, gsq[2][0][:])              # DVE
        nc.scalar.activation(grinv[1][:], gsq[1][0][:],
                             AF.Abs_reciprocal_sqrt)
        nc.scalar.activation(wt[1][:], oh_sb[:, C:2 * C], AF.Copy,
                             scale=grinv[1][:])
        stt_sq(3, None, gsq[3][0][:])              # DVE
        nc.scalar.activation(grinv[2][:], gsq[2][0][:],
                             AF.Abs_reciprocal_sqrt)
        nc.vector.tensor_scalar_mul(wt[2][:], oh_sb[:, 2 * C:3 * C],
                                    grinv[2][:])
        nc.scalar.activation(grinv[3][:], gsq[3][0][:],
                             AF.Abs_reciprocal_sqrt)
        nc.scalar.activation(wt[3][:], oh_sb[:, 3 * C:4 * C], AF.Copy,
                             scale=grinv[3][:])

        for t in range(NT):
            for h in range(2):
                nc.tensor.matmul(
                    pa[h][:],
                    wt[t][:],
                    seg(t, h),
                    start=(t == 0),
                    stop=(t == NT - 1),
                )
        # Drain in halves; pa is split per-half so the h0 copy does not
        # falsely wait on the h1 matmul (whole-tile dep tracking).
        o = const_pool.tile([C, D], BF16, tag="o")
        QB1 = HB // 2
        for h in range(2):
            for q in range(2):
                qs = slice(h * HB + q * QB1, h * HB + (q + 1) * QB1)
                qp = slice(q * QB1, (q + 1) * QB1)
                nc.vector.tensor_copy(o[:, qs], pa[h][:, qp])
                queue = nc.sync if h == 0 else nc.scalar
                queue.dma_start(atp[:, qs], o[:, qs])

    nc.compile()
    return nc


def _build_phase2():
    nc = bacc.Bacc(
        "TRN2", target_bir_lowering=False, debug=False, num_devices=NCORES
    )
    a = nc.dram_tensor("a", [P, DC * C], BF16, kind="ExternalInput").ap()
    fxt = nc.dram_tensor("fxt", [P, DC * MS], BF16, kind="ExternalInput").ap()
    out = nc.dram_tensor("out", [C, MS], BF16, kind="ExternalOutput").ap()

    with tile.TileContext(nc) as tc, ExitStack() as ctx:
        const_pool = ctx.enter_context(tc.tile_pool(name="const", bufs=1))
        sq_pool = ctx.enter_context(tc.tile_pool(name="sqp", bufs=4))
        st_pool = const_pool
        os_pool = const_pool
        psO = ctx.enter_context(tc.tile_pool(name="psO", bufs=1, space="PSUM"))
        psF = psO
        psJ = psO

        a_sb = const_pool.tile([P, DC * C], BF16, tag="a")
        fxt_sb = const_pool.tile([P, DC * MS], BF16, tag="fxt")

        def chunk_ap(k):
            return fxt_sb[:, k * MS:(k + 1) * MS]

        def pair_dma(q, k0):
            q.dma_start(
                fxt_sb[:, k0 * MS:(k0 + 2) * MS].rearrange(
                    "p (t m) -> p t m", t=2),
                fxt[:, k0 * MS:(k0 + 2) * MS].rearrange(
                    "p (t m) -> p t m", t=2),
            )

        # Baseline-measured DMA byte schedule (singles + 3D pairs +
        # trailing halves), with the chunk labels arranged so the LAST
        # arrival is c6 (direct-folded, split-squared) instead of c7:
        #   sync:   c0, [c1 c2], [c3 c4], c6h0, c6h1
        #   scalar: a, c5, c7
        nc.sync.dma_start(chunk_ap(0), fxt[:, 0:MS])
        pair_dma(nc.sync, 1)          # c1, c2
        nc.scalar.dma_start(chunk_ap(7), fxt[:, 7 * MS:8 * MS])
        pair_dma(nc.sync, 3)          # c3, c4
        nc.scalar.dma_start(a_sb[:], a[:, :])
        nc.scalar.dma_start(chunk_ap(5), fxt[:, 5 * MS:6 * MS])
        nc.sync.dma_start(fxt_sb[:, 6 * MS:6 * MS + HB],
                          fxt[:, 6 * MS:6 * MS + HB])
        nc.sync.dma_start(fxt_sb[:, 6 * MS + HB:7 * MS],
                          fxt[:, 6 * MS + HB:7 * MS])

        ones_sb = const_pool.tile([P, C], F16, tag="ones")
        nc.gpsimd.memset(ones_sb[:], 1.0)
        with tc.high_priority():
            dumm = st_pool.tile([1, 1], F32, tag="dumm")
            nc.gpsimd.memset(dumm[:], 1.0)
            dumm2 = st_pool.tile([1, 1], F32, tag="dumm2")
            nc.scalar.activation(dumm2[:], dumm[:], AF.Square)
            nc.scalar.activation(dumm2[:], dumm[:], AF.Abs_reciprocal_sqrt)
            # Narrow PE keep-alive matmuls (see phase 1).
            jmov = const_pool.tile([P, C], BF16, tag="jmov")
            nc.gpsimd.memset(jmov[:], 1.0)
            pj = psJ.tile([1, C], F32, tag="junkp", name="pj")
            for i in range(P2_JUNK):
                nc.tensor.matmul(pj[:], jmov[:, 0:1], jmov[:], start=True,
                                 stop=True)

        # po/pf split per 512-wide half (separate PSUM banks) so reads
        # of one half never falsely serialize against writes of the
        # other (whole-tile dep tracking).
        po = [psO.tile([C, HB], F32, tag=f"ot{h}", name=f"po{h}")
              for h in range(2)]
        pf = [psF.tile([C, HB], F32, tag=f"fs{h}", name=f"pf{h}")
              for h in range(2)]
        # Square accumulators grouped by arrival: sacc0={0,1,2} and
        # sacc1={3,4} (adds on DVE), sacc2={5,7} (add on DVE). The
        # last-arriving chunk c6 skips accumulation: its square runs as
        # h0->DVE / h1->ACT in parallel and folds straight into pf.
        sacc = [
            const_pool.tile([P, MS], F16, tag=f"sacc{j}", name=f"sacc{j}")
            for j in range(2)
        ]
        GROUP = {0: 0, 1: 0, 2: 0, 3: 0, 4: 1, 5: 1}
        INIT = {0: True, 4: True}
        ACT_SQ = (1, 3, 5)
        SQ_ORDER = (0, 1, 2, 3, 4, 5)
        for k in SQ_ORDER:
            acc = sacc[GROUP[k]]
            dst = acc if INIT.get(k) else sq_pool.tile(
                [P, MS], F16, tag="sq", name=f"sq{k}")
            if k in ACT_SQ:
                nc.scalar.activation(dst[:], chunk_ap(k), AF.Square)
            else:
                nc.vector.tensor_mul(dst[:], chunk_ap(k), chunk_ap(k))
            if not INIT.get(k):
                nc.vector.tensor_add(acc[:], acc[:], dst[:])
        # c7 and c6 skip the accumulator: their squares fold straight
        # into pf, so no serial DVE add sits between their arrival and
        # the fold. c7 squares on ACT (it arrives early, scalar-first);
        # c6 (last arrival) splits h0->DVE / h1->ACT.
        sq7 = sq_pool.tile([P, MS], F16, tag="sq7", name="sq7")
        nc.scalar.activation(sq7[:], chunk_ap(7), AF.Square)
        sq6 = sq_pool.tile([P, MS], F16, tag="sq6", name="sq6")
        nc.vector.tensor_mul(sq6[:, 0:HB], chunk_ap(6)[:, 0:HB],
                             chunk_ap(6)[:, 0:HB])
        nc.scalar.activation(sq6[:, HB:MS], chunk_ap(6)[:, HB:MS],
                             AF.Square)

        # Sim stream in order 0..5,7,6 (PSUM accumulation commutes); the
        # sacc folds interleave at the k==5/k==7 slots and c6's direct
        # fold is the only norm work after the last sim matmul.
        frinv = const_pool.tile([C, MS], F32, tag="frinv")
        ot_sb = os_pool.tile([C, MS], BF16, tag="otsb")
        MM_ORDER = (0, 1, 2, 3, 4, 5, 7, 6)
        for i, k in enumerate(MM_ORDER):
            chunk = chunk_ap(k)
            for h in range(2):
                nc.tensor.matmul(
                    po[h][:],
                    a_sb[:, k * C:(k + 1) * C],
                    chunk[:, h * HB:(h + 1) * HB],
                    start=(i == 0),
                    stop=(i == DC - 1),
                )
            if k == 5:
                for h in range(2):
                    hs = slice(h * HB, (h + 1) * HB)
                    nc.tensor.matmul(pf[h][:], ones_sb[:], sacc[0][:, hs],
                                     start=True, stop=False)
            if k == 7:
                for h in range(2):
                    hs = slice(h * HB, (h + 1) * HB)
                    nc.tensor.matmul(pf[h][:], ones_sb[:], sq7[:, hs],
                                     start=False, stop=False)
        for h in range(2):
            hs = slice(h * HB, (h + 1) * HB)
            nc.tensor.matmul(pf[h][:], ones_sb[:], sacc[1][:, hs],
                             start=False, stop=False)
        QB = HB // 2
        for h in range(2):
            hs = slice(h * HB, (h + 1) * HB)
            nc.tensor.matmul(pf[h][:], ones_sb[:], sq6[:, hs],
                             start=False, stop=True)
            for q in range(2):
                qs = slice(h * HB + q * QB, h * HB + (q + 1) * QB)
                qp = slice(q * QB, (q + 1) * QB)
                nc.scalar.activation(frinv[:, qs], pf[h][:, qp],
                                     AF.Abs_reciprocal_sqrt)
                nc.vector.tensor_mul(ot_sb[:, qs], po[h][:, qp],
                                     frinv[:, qs])
                queue = nc.sync if h == 0 else nc.scalar
                queue.dma_start(out[:, qs], ot_sb[:, qs])

    nc.compile()
    return nc


def _get_ncs():
    if "nc1" not in _CACHE:
        _CACHE["nc1"] = _build_phase1()
        _CACHE["nc2"] = _build_phase2()
    return _CACHE["nc1"], _CACHE["nc2"]


class _FakeResult:
    def __init__(self, results):
        self.results = results
        self.exec_time_ns = None
        self.instructions_and_trace = None


def _make_runner(nc):
    """One persistently-jitted shard_map executable for this Bass module.

    run_bass_via_pjrt rebuilds its jit closure per call, which retraces and
    re-lowers the HLO every invocation (~3 s/launch of host time). Caching
    the jitted callable keeps warmed kernel() calls fast; the device-side
    NEFF and its execution are identical.
    """
    import jax
    import numpy as _np

    bass2jax.install_neuronx_cc_hook()
    Mesh = bass2jax.Mesh
    PartitionSpec = bass2jax.PartitionSpec
    shard_map = bass2jax.shard_map

    partition_name = (
        nc.partition_id_tensor.name if nc.partition_id_tensor else None
    )
    in_names, out_names, out_avals, zero_shapes = [], [], [], []
    for alloc in nc.m.functions[0].allocations:
        if not isinstance(alloc, mybir.MemoryLocationSet):
            continue
        name = alloc.memorylocations[0].name
        if alloc.kind == "ExternalInput":
            if name != partition_name:
                in_names.append(name)
        elif alloc.kind == "ExternalOutput":
            shape = tuple(alloc.tensor_shape)
            dtype = mybir.dt.np(alloc.dtype)
            out_avals.append(jax.core.ShapedArray(shape, dtype))
            out_names.append(name)
            zero_shapes.append((shape, dtype))
    n_params = len(in_names)
    all_in = list(in_names) + list(out_names)
    if partition_name is not None:
        all_in.append(partition_name)
    donate = tuple(range(n_params, n_params + len(out_names)))

    def _body(*args):
        operands = list(args)
        if partition_name is not None:
            operands.append(bass2jax.partition_id_tensor())
        outs = bass2jax._bass_exec_p.bind(
            *operands,
            out_avals=tuple(out_avals),
            in_names=tuple(all_in),
            out_names=tuple(out_names),
            lowering_input_output_aliases=(),
            sim_require_finite=True,
            sim_require_nnan=True,
            nc=nc,
        )
        return tuple(outs)

    devices = jax.devices()[:NCORES]
    mesh = Mesh(_np.asarray(devices), ("core",))
    nspec = n_params + len(out_names)
    sharded = jax.jit(
        shard_map(
            _body, mesh=mesh,
            in_specs=(PartitionSpec("core"),) * nspec,
            out_specs=(PartitionSpec("core"),) * len(out_names),
            check_rep=False,
        ),
        donate_argnums=donate,
        keep_unused=True,
    )

    def runner(in_maps):
        concat_in = [
            _np.concatenate([_np.asarray(m[name]) for m in in_maps], axis=0)
            for name in in_names
        ]
        concat_zeros = [
            _np.zeros((NCORES * s[0], *s[1:]), dt) for s, dt in zero_shapes
        ]
        out_arrs = sharded(*concat_in, *concat_zeros)
        return _FakeResult([
            {
                name: _np.asarray(out_arrs[i]).reshape(
                    NCORES, *out_avals[i].shape
                )[c]
                for i, name in enumerate(out_names)
            }
            for c in range(NCORES)
        ])

    return runner


def _get_runners():
    if "run1" not in _CACHE:
        nc1, nc2 = _get_ncs()
        _CACHE["run1"] = _make_runner(nc1)
        _CACHE["run2"] = _make_runner(nc2)
    return _CACHE["run1"], _CACHE["run2"]


def _tile_rows_flat(arr, ntiles):
    """[ntiles*128, F] -> [128, ntiles*F] with [p, t*F+f] = arr[t*128+p, f]."""
    f = arr.shape[1]
    return np.ascontiguousarray(
        arr.reshape(ntiles, P, f).transpose(1, 0, 2).reshape(P, ntiles * f)
    )


def run(gS, fX, trainTarget, nClasses, trace=False, **spmd_kwargs):
    nc1, nc2 = _get_ncs()
    gS = np.asarray(gS, dtype=np.float32).astype(BF16NP)
    fX = np.asarray(fX, dtype=np.float32).astype(BF16NP)
    tt = np.asarray(trainTarget).astype(np.int64).ravel()
    nc_classes = int(np.asarray(nClasses))
    assert nc_classes == C and gS.shape == (N, D) and fX.shape == (M, D)

    oh = np.zeros((N, C), dtype=BF16NP)
    oh[np.arange(N), tt] = 1.0

    in_maps1 = []
    for i in range(NCORES):
        gsl = gS[i * NS:(i + 1) * NS]
        osl = oh[i * NS:(i + 1) * NS]
        in_maps1.append(
            {"gs": _tile_rows_flat(gsl, NT), "oh": _tile_rows_flat(osl, NT)}
        )
    if trace or spmd_kwargs:
        res1 = run_bass_kernel_spmd(
            nc1, in_maps1, core_ids=list(range(NCORES)), trace=trace,
            **spmd_kwargs
        )
    else:
        res1 = _get_runners()[0](in_maps1)
    # gather-reduce the partial A.T's, retile to [128, 8*64] bf16
    at = np.zeros((C, D), dtype=np.float32)
    for i in range(NCORES):
        at += res1.results[i]["atp"].astype(np.float32)
    a_tiled = _tile_rows_flat(np.ascontiguousarray(at.T.astype(BF16NP)), DC)

    in_maps2 = []
    for i in range(NCORES):
        sl = fX[i * MS:(i + 1) * MS]                       # [MS, D] bf16
        fxt_tiled = np.ascontiguousarray(
            sl.T.reshape(DC, P, MS).transpose(1, 0, 2).reshape(P, DC * MS)
        )
        in_maps2.append({"a": a_tiled, "fxt": fxt_tiled})
    if trace or spmd_kwargs:
        res2 = run_bass_kernel_spmd(
            nc2, in_maps2, core_ids=list(range(NCORES)), trace=trace,
            **spmd_kwargs
        )
    else:
        res2 = _get_runners()[1](in_maps2)
    outs = [
        np.ascontiguousarray(res2.results[i]["out"].T).astype(np.float32)
        for i in range(NCORES)
    ]
    full = np.concatenate(outs, axis=0)
    return full, (res1, res2)


def _axon_reset():
    """Recover a wedged exec unit (NRT_EXEC_UNIT_UNRECOVERABLE) left over
    from an earlier crashed run; no-op on a healthy device."""
    try:
        import ctypes
        import jax

        lib = ctypes.CDLL("/opt/axon/libaxon_pjrt.so")
        lib.axon_reset.restype = ctypes.c_int64
        jax.devices()
        lib.axon_reset(ctypes.c_int32(0))
        jax.clear_caches()
        _CACHE.pop("run1", None)
        _CACHE.pop("run2", None)
    except Exception:
        pass


def kernel(gS, fX, trainTarget, nClasses):
    try:
        full, _ = run(gS, fX, trainTarget, nClasses)
    except Exception:
        _axon_reset()
        full, _ = run(gS, fX, trainTarget, nClasses)
    return full
